# revision 30
# baseline (speedup 1.0000x reference)
"""Multi-headed attention kernel for 8 Trainium2 NeuronCores.

Problem: B=4, S=2048, E=1024, H=16, D=64 (torch-convention Linears, no bias
on q/k/v, bias on output projection).

Sharding: core c handles (batch b = c//2, query half sh = c%2).  Each core
computes Q for its 1024 query rows, K/V for the full 2048 keys of its batch
(duplicated across the pair of cores sharing a batch -- cheaper than any
cross-core collective), all 16 heads of attention for its rows, and the
output projection + bias.  Zero collectives.

Layout (feature dim on partitions; scores computed transposed):
  qT[f, q]  = sum_e WqT[e, f] * XT[e, q]          kT[f, s] likewise
  V[s, f]   = sum_e XT[e, s-chunk] * WvT[e, f]    (natural layout)
  scoresT[k, q] = sum_d kT[h*64+d, kc] * qT[h*64+d, q]    (K=64 matmuls,
      head pair packed in complementary PE row groups)
  EX = exp(scoresT / 8)            (ACT engine, PSUM -> SBUF bf16)
  ctxT_aug[m, q] = sum_k Vaug[k, h*65+m] * EX[k, q]   m in 0..64; V carries
      a ones column per head, so row 64 of the accumulation = softmax
      denominators (ones-column trick, M=65 matmuls)
  ctxT_norm = ctxT * bcast(1/denom)   (recip on DVE over [1,512], then the
      idle GpSimd engine broadcasts it across partitions -- no PE involved)
  out[s, e] = sum_f ctxT_norm[f, s-chunk] * WoT[f, e] + bo   (bias is
      host-tiled to [128, E] and added by the DVE during the PSUM->SBUF
      copy -- no K=1 bias matmuls on the PE)

Scheduling notes (HW-trace driven):
  * LDWEIGHTS only ping-pongs into the background weight buffer when the
    tile config (size/position) matches the in-flight matmul; any config
    switch stalls ~100ns until the array drains.  The kc loop is therefore
    batched j=2: [pair kc, pair kc+1] then [ctx x4], so the tiled->full
    transitions amortize over two kc of work and same-config neighbors
    stream back-to-back at the N/2.4 rate.
  * The exp stream on the Scalar engine costs (1024+352)/1.2 = 1147ns per
    kc -- nearly co-critical with the PE.  The per-engine program order is
    fixed at compile time, so filler projections are explicitly interleaved
    between kc batches (one ~1.75us group slice per batch) instead of
    emitted in a blob at pair boundaries; the boundary blobs in the v1
    schedule produced 13us exp holes while ~50 queued fillers drained.
  * Softmax normalization runs immediately at unit end (den extraction and
    unnormalized-ctxt copy at high priority to release the ctx PSUM ring),
    with the 1/den broadcast on GpSimd instead of K=1 PE matmuls.
  PSUM: scores 2x[128,1024] (4) + ctx 2x[65,512] (2) + proj 2x[128,512]
  (2) = 8 banks.
"""

import os

import numpy as np
import ml_dtypes

import concourse.bass as bass
from concourse import bacc
import concourse.mybir as mybir
import concourse.tile as tile
from concourse.bass_utils import run_bass_kernel_spmd

B, S, E, H = 4, 2048, 1024, 16
D = E // H  # 64
P = 128
SL = S // 2     # local query rows per core (1024)
NCORES = 8
EC = E // P     # 8 e-chunks
FC = E // P     # 8 feature chunks
SC = S // P     # 16 s-chunks (V natural layout)
KC = S // P     # 16 key chunks (scores partition dim)
QB = SL // 512  # 2 query blocks of 512

F32 = mybir.dt.float32
BF16 = mybir.dt.bfloat16
EXPF = mybir.ActivationFunctionType.Exp
NPBF = ml_dtypes.bfloat16

_CACHE = {}


def build():
    nc = bacc.Bacc(
        "TRN2",
        target_bir_lowering=False,
        debug=False,
        num_devices=NCORES,
    )

    xt_d = nc.dram_tensor("xt", [E, S], BF16, kind="ExternalInput").ap()
    # wq2/wk2 are host-pretiled: row fc*128+p, col ec*128+c  =  W.T[ec*128+p,
    # fc*128+c], so one contiguous [128, E] DMA delivers all 8 lhsT slices
    # for feature chunk fc.
    wq2_d = nc.dram_tensor("wq2", [E, E], BF16, kind="ExternalInput").ap()
    wk2_d = nc.dram_tensor("wk2", [E, E], BF16, kind="ExternalInput").ap()
    wvt_d = nc.dram_tensor("wvt", [E, E], BF16, kind="ExternalInput").ap()
    wot_d = nc.dram_tensor("wot", [E, E], BF16, kind="ExternalInput").ap()
    bo_d = nc.dram_tensor("bo", [P, E], F32, kind="ExternalInput").ap()  # host-tiled bias
    out_d = nc.dram_tensor("out", [SL, E], F32, kind="ExternalOutput").ap()

    with tile.TileContext(nc) as tc:
     with tc.tile_pool(name="persist", bufs=1) as persist:
        qt_sb = persist.tile([P, FC, SL], BF16, tag="qt")
        kt_sb = persist.tile([P, FC, S], BF16, tag="kt")
        DA = D + 1  # head dim + ones column
        vaug_sb = persist.tile([P, SC, H * DA], BF16, tag="vaug")
        vview = vaug_sb.rearrange("p c (h d) -> p c h d", d=DA)
        nc.vector.memset(vview[:, :, :, D : D + 1], 1.0)
        ctxt_sb = persist.tile([P, FC, SL], BF16, tag="ctxt")

        ones_bf = persist.tile([1, P], BF16, tag="ones_bf")   # dummy-act input
        nc.vector.memset(ones_bf[:], 1.0)

        from contextlib import ExitStack

        with (
            tc.tile_pool(name="wvp", bufs=8) as wvpool,
            tc.tile_pool(name="wqkp", bufs=5) as wqkpool,
            tc.tile_pool(name="expp", bufs=10) as exppool,
            tc.tile_pool(name="smallp", bufs=4) as smallpool,
        ):
            _xstack = ExitStack()
            _ostack = ExitStack()
            xpool = _xstack.enter_context(tc.tile_pool(name="xp", bufs=1))
            x_sb = xpool.tile([P, EC, S], BF16, tag="x")
            wv = []
            wot_t = []
            bo_sb = persist.tile([P, E], F32, tag="bo")
            outpool_box = [None]
            partpool_box = [None]

            def load_wfc(w_dram, fc):
                """One [128, E] tile holding all 8 lhsT slices for chunk fc."""
                t = wqkpool.tile([P, E], BF16, tag="wqk", name="wqk")
                nc.sync.dma_start(out=t[:], in_=w_dram[fc * P : (fc + 1) * P, :])
                return t

            # ---- projection group emitters (8 accumulating MMs + 1 cast) ----
            def q_group(pool, wq_t, fc, qb):
                ps = pool.tile([P, 512], F32, tag="pj", name="pj")
                for ec in range(EC):
                    nc.tensor.matmul(
                        ps[:],
                        wq_t[:, ec * P : (ec + 1) * P],
                        x_sb[:, ec, qb * 512 : qb * 512 + 512],
                        start=(ec == 0),
                        stop=(ec == EC - 1),
                    )
                nc.vector.tensor_copy(
                    out=qt_sb[:, fc, qb * 512 : qb * 512 + 512], in_=ps[:]
                )

            def k_group(pool, wk_t, fc, kb):
                ps = pool.tile([P, 512], F32, tag="pj", name="pj")
                for ec in range(EC):
                    nc.tensor.matmul(
                        ps[:],
                        wk_t[:, ec * P : (ec + 1) * P],
                        x_sb[:, ec, kb * 512 : kb * 512 + 512],
                        start=(ec == 0),
                        stop=(ec == EC - 1),
                    )
                nc.vector.tensor_copy(
                    out=kt_sb[:, fc, kb * 512 : kb * 512 + 512], in_=ps[:]
                )

            def v_group(pool, sc, fb):
                ps = pool.tile([P, 512], F32, tag="pj", name="pj")
                for ec in range(EC):
                    nc.tensor.matmul(
                        ps[:],
                        x_sb[:, ec, sc * P : (sc + 1) * P],
                        wv[ec][:, fb * 512 : fb * 512 + 512],
                        start=(ec == 0),
                        stop=(ec == EC - 1),
                    )
                vv = vaug_sb[:, sc, :].rearrange("p (h d) -> p h d", d=DA)
                nc.vector.tensor_copy(
                    out=vv[:, fb * 8 : (fb + 1) * 8, 0:D],
                    in_=ps.rearrange("p (h d) -> p h d", d=D),
                )

            # out-proj is split so only the fcc=7 term waits on the last
            # norm: the fcc 0..6 partial (7 MMs) is dep-ready two units
            # early and runs as ordinary filler, cast to SBUF bf16 with the
            # bias pre-added; the tail is then one matmul + one vector add
            # per (sc, eb) instead of a 23us post-exp PE blob.
            def o_partial(pool, sc, eb, part):
                ps = pool.tile([P, 512], F32, tag="pj", name="pj")
                for fcc in range(FC - 1):
                    nc.tensor.matmul(
                        ps[:],
                        ctxt_sb[:, fcc, sc * P : (sc + 1) * P],
                        wot_t[fcc][:, eb * 512 : eb * 512 + 512],
                        start=(fcc == 0),
                        stop=(fcc == FC - 2),
                    )
                nc.vector.scalar_tensor_tensor(
                    out=part[:, eb * 512 : eb * 512 + 512],
                    in0=ps[:],
                    scalar=1.0,
                    in1=bo_sb[:, eb * 512 : eb * 512 + 512],
                    op0=mybir.AluOpType.mult,
                    op1=mybir.AluOpType.add,
                )

            def o_final(pool, sc, eb, part, ot):
                ps = pool.tile([P, 512], F32, tag="pj", name="pj")
                nc.tensor.matmul(
                    ps[:],
                    ctxt_sb[:, FC - 1, sc * P : (sc + 1) * P],
                    wot_t[FC - 1][:, eb * 512 : eb * 512 + 512],
                    start=True,
                    stop=True,
                )
                nc.vector.scalar_tensor_tensor(
                    out=ot[:, eb * 512 : eb * 512 + 512],
                    in0=ps[:],
                    scalar=1.0,
                    in1=part[:, eb * 512 : eb * 512 + 512],
                    op0=mybir.AluOpType.mult,
                    op1=mybir.AluOpType.add,
                )
                if eb == 1:
                    nc.sync.dma_start(
                        out=out_d[sc * P : (sc + 1) * P, :], in_=ot[:]
                    )

            # ---------------- upfront: just enough for pair 0 ----------------
            # W chunk-0 tiles go on the Sync DMA queue; X streams across the
            # GpSimd/Scalar/Vector DMA queues, first-half (hx=0) chunks
            # first so the qb0/kb0-1 projection groups complete after 8
            # transfers instead of 15; Wv after X on Sync.
            wq_sl = load_wfc(wq2_d, 0)
            wk_sl = load_wfc(wk2_d, 0)
            for hx in range(2):
                for ec in range(EC):
                    eng = nc.gpsimd if ec % 2 == 0 else nc.scalar
                    eng.dma_start(
                        out=x_sb[:, ec, hx * 1024 : (hx + 1) * 1024],
                        in_=xt_d[ec * P : (ec + 1) * P, hx * 1024 : (hx + 1) * 1024],
                    )
            # Dummy exp preloads the ACT table (~2.7us) during the DMA phase;
            # emitted AFTER the X dma_starts so it doesn't delay the Scalar
            # queue's descriptor generation.
            dummy_act = smallpool.tile([1, 16], BF16, tag="recb", name="recb")
            nc.scalar.activation(dummy_act[:], ones_bf[0:1, 0:16], EXPF)
            for ec in range(EC):
                t = wvpool.tile([P, E], BF16, tag="wv", name="wv")
                nc.sync.dma_start(out=t[:], in_=wvt_d[ec * P : (ec + 1) * P, :])
                wv.append(t)
            with tc.tile_pool(name="psum_u", bufs=6, space="PSUM") as psum_u:
                # advance all 6 Q/K accumulation groups together per arriving
                # X chunk: each 1.6us chunk DMA feeds ~1.6us of matmuls, so
                # the PE ramps with the DMA stream instead of stalling on the
                # last chunk of each group.
                psq = [
                    psum_u.tile([P, 512], F32, tag="pj", name="pj")
                    for _ in range(QB)
                ]
                psk = [
                    psum_u.tile([P, 512], F32, tag="pj", name="pj")
                    for _ in range(4)
                ]
                # wave 1: the four groups needing only first-half X chunks
                # (qb0/qb1/kb0/kb1 all live in columns 0:1024), interleaved
                # per arriving hx=0 chunk; kb2/kb3 form wave 2 behind the
                # hx=1 transfers so the compile-time PE stream never stalls
                # on a second-half chunk mid-wave.
                for ec in range(EC):
                    for qb in range(QB):
                        nc.tensor.matmul(
                            psq[qb][:],
                            wq_sl[:, ec * P : (ec + 1) * P],
                            x_sb[:, ec, qb * 512 : qb * 512 + 512],
                            start=(ec == 0),
                            stop=(ec == EC - 1),
                        )
                    for kb in range(2):
                        nc.tensor.matmul(
                            psk[kb][:],
                            wk_sl[:, ec * P : (ec + 1) * P],
                            x_sb[:, ec, kb * 512 : kb * 512 + 512],
                            start=(ec == 0),
                            stop=(ec == EC - 1),
                        )
                for qb in range(QB):
                    nc.vector.tensor_copy(
                        out=qt_sb[:, 0, qb * 512 : qb * 512 + 512], in_=psq[qb][:]
                    )
                for kb in range(2):
                    nc.vector.tensor_copy(
                        out=kt_sb[:, 0, kb * 512 : kb * 512 + 512], in_=psk[kb][:]
                    )
                # V first (hx=0-only deps) so wave 2 overlaps the hx=1 DMAs
                for sc in range(4):
                    v_group(psum_u, sc, 0)
                for sc in range(4, 6):
                    v_group(psum_u, sc, 0)
                # wave 2: kb2/kb3 (second-half X chunks)
                for ec in range(EC):
                    for kb in range(2, 4):
                        nc.tensor.matmul(
                            psk[kb][:],
                            wk_sl[:, ec * P : (ec + 1) * P],
                            x_sb[:, ec, kb * 512 : kb * 512 + 512],
                            start=(ec == 0),
                            stop=(ec == EC - 1),
                        )
                for kb in range(2, 4):
                    nc.vector.tensor_copy(
                        out=kt_sb[:, 0, kb * 512 : kb * 512 + 512], in_=psk[kb][:]
                    )

            # ---------------- main loop: (fc, qb) units, kc batches of 2 ----
            with (
                tc.tile_pool(name="psum_sc", bufs=2, space="PSUM") as psum_sc,
                tc.tile_pool(name="psum_cx", bufs=2, space="PSUM") as psum_cx,
                tc.tile_pool(name="psum_pj", bufs=2, space="PSUM") as psum_pj,
            ):
                # ---- static filler schedule: unit (fc,qb) -> list of
                # closures, one emitted after each kc batch (8 slots/unit).
                # K(f)/Q(f) prepped during fc=f-1; V fb0 upfront+unit(0,0);
                # V fb1 during fc in 1..3; out-proj sc0-3 during (7,1).
                wnames = {}

                def _load_k(f):
                    def go():
                        wnames[("k", f)] = load_wfc(wk2_d, f)
                    return go

                def _load_q(f):
                    def go():
                        wnames[("q", f)] = load_wfc(wq2_d, f)
                    return go

                def _k(f, kb):
                    def go():
                        k_group(psum_pj, wnames[("k", f)], f, kb)
                    return go

                def _q(f, qb):
                    def go():
                        q_group(psum_pj, wnames[("q", f)], f, qb)
                    return go

                def _v(sc, fb):
                    def go():
                        v_group(psum_pj, sc, fb)
                    return go

                def _wo_prefetch():
                    def go():
                        for fcc in range(FC):
                            t = wvpool.tile([P, E], BF16, tag="wv", name="wv")
                            nc.sync.dma_start(
                                out=t[:], in_=wot_d[fcc * P : (fcc + 1) * P, :]
                            )
                            wot_t.append(t)
                        nc.sync.dma_start(out=bo_sb[:], in_=bo_d[:])
                    return go

                parts = {}

                def _op(sc, eb):
                    def go():
                        if sc not in parts:
                            parts[sc] = partpool_box[0].tile(
                                [P, E], BF16, tag="part", name="part"
                            )
                        o_partial(psum_pj, sc, eb, parts[sc])
                    return go

                def _of(sc, eb, ot_box):
                    def go():
                        if eb == 0:
                            ot_box[0] = outpool_box[0].tile(
                                [P, E], F32, tag="out", name="out"
                            )
                        o_final(psum_pj, sc, eb, parts[sc], ot_box[0])
                    return go

                sched = {}
                for fc in range(FC):
                    for qb in range(QB):
                        sched[(fc, qb)] = []
                u = sched
                # unit (0,0): V sc6..15 inline (2 per batch, 4-chunk lead),
                # then K(1) kb0/kb1 + loads
                for b in range(5):
                    u[(0, 0)] += [_v(2 * b + 6, 0), _v(2 * b + 7, 0)]
                u[(0, 0)] += [_load_k(1), _k(1, 0), _load_q(1), _k(1, 1)]
                # unit (0,1): Q(1,*) + K(1) kb2/3 + start V fb1
                u[(0, 1)] += [
                    _q(1, 0), _k(1, 2), _k(1, 3), _q(1, 1),
                    _v(0, 1), _v(1, 1), _v(2, 1), _v(3, 1),
                ]
                vn = 4  # next fb1 V chunk
                for fc in range(1, FC - 1):
                    nf = fc + 1
                    a = [_load_k(nf), _k(nf, 0), _load_q(nf), _k(nf, 1)]
                    bl = [_q(nf, 0), _k(nf, 2), _k(nf, 3), _q(nf, 1)]
                    if fc < 4:
                        a += [_v(vn, 1), _v(vn + 1, 1)]
                        bl += [_v(vn + 2, 1), _v(vn + 3, 1)]
                        vn += 4
                    if fc == 4:
                        a = [_wo_prefetch()] + a
                    u[(fc, 0)] += a
                    u[(fc, 1)] += bl
                # unit (7,0): fcc0-6 partials for qb0's rows; unit (7,1):
                # partials for qb1's rows, then fcc=7 finals for qb0's rows
                ot_boxes = [[None] for _ in range(8)]
                for sc in range(4):
                    u[(7, 0)] += [_op(sc, 0), _op(sc, 1)]
                for sc in range(4, 8):
                    u[(7, 1)] += [_op(sc, 0), _op(sc, 1)]
                for sc in range(4):
                    u[(7, 1)] += [_of(sc, 0, ot_boxes[sc]), _of(sc, 1, ot_boxes[sc])]

                for fc in range(FC):
                    hA, hB = 2 * fc, 2 * fc + 1
                    for qb in range(QB):
                        fill = list(sched[(fc, qb)])
                        ctx_ps = {
                            hh: psum_cx.tile([DA, 512], F32, tag="ctx", name="ctx")
                            for hh in (0, 1)
                        }
                        for kcb in range(KC // 2):
                            with tc.high_priority(offset=1 << 20):
                                exs = []
                                for j2 in (0, 1):
                                    kc = 2 * kcb + j2
                                    sc_ps = psum_sc.tile(
                                        [P, 1024], F32, tag="sc", name="sc"
                                    )
                                    for hh in (0, 1):
                                        po = hh * D
                                        nc.tensor.matmul(
                                            sc_ps[:, hh * 512 : hh * 512 + 512],
                                            kt_sb[
                                                po : po + D,
                                                fc,
                                                kc * P : (kc + 1) * P,
                                            ],
                                            qt_sb[
                                                po : po + D,
                                                fc,
                                                qb * 512 : qb * 512 + 512,
                                            ],
                                            start=True,
                                            stop=True,
                                        )
                                    ex = exppool.tile(
                                        [P, 1024], BF16, tag="exp", name="exp"
                                    )
                                    nc.scalar.activation(
                                        ex[:], sc_ps[:], EXPF, scale=0.125
                                    )
                                    exs.append(ex)
                                for j2 in (0, 1):
                                    kc = 2 * kcb + j2
                                    for hh, h in ((0, hA), (1, hB)):
                                        nc.tensor.matmul(
                                            ctx_ps[hh][0:DA, :],
                                            vaug_sb[:, kc, h * DA : (h + 1) * DA],
                                            exs[j2][:, hh * 512 : hh * 512 + 512],
                                            start=(kc == 0),
                                            stop=(kc == KC - 1),
                                        )
                            # filler slices (normal priority): drain the
                            # unit's queue evenly across its 8 batches
                            nb = KC // 2 - kcb
                            npop = (len(fill) + nb - 1) // nb if fill else 0
                            for _ in range(npop):
                                fill.pop(0)()

                        # ---- normalize immediately: den + unnorm copy at
                        # high priority (releases the cx ring), recip/bcast/
                        # mul off the PE.
                        qsl = slice(qb * 512, qb * 512 + 512)
                        dens = []
                        with tc.high_priority(offset=1 << 20):
                            for hh in (0, 1):
                                den = smallpool.tile(
                                    [1, 512], F32, tag="den", name="den"
                                )
                                nc.vector.tensor_copy(
                                    out=den[:], in_=ctx_ps[hh][D : D + 1, :]
                                )
                                dens.append(den)
                            for hh in (0, 1):
                                nc.vector.tensor_copy(
                                    out=ctxt_sb[hh * D : (hh + 1) * D, fc, qsl],
                                    in_=ctx_ps[hh][0:D, :],
                                )
                        rec_all = smallpool.tile(
                            [P, 512], BF16, tag="recall", name="recall"
                        )
                        for hh in (0, 1):
                            rec_f = smallpool.tile(
                                [1, 512], F32, tag="recf", name="recf"
                            )
                            nc.vector.reciprocal_approx_fast(
                                out=rec_f[:], in_=dens[hh][:]
                            )
                            rec_b = smallpool.tile(
                                [1, 512], BF16, tag="recb", name="recb"
                            )
                            nc.vector.tensor_copy(out=rec_b[:], in_=rec_f[:])
                            # partition_broadcast's write mask uses absolute
                            # partition indices (cpu_id*16+lane < channels),
                            # so it can only target base-partition-0 slices;
                            # head B goes through a base-0 tile + DVE copy.
                            if hh == 0:
                                nc.gpsimd.partition_broadcast(
                                    rec_all[0:D, :], rec_b[:]
                                )
                            else:
                                rtmp = smallpool.tile(
                                    [D, 512], BF16, tag="rtmp", name="rtmp"
                                )
                                nc.gpsimd.partition_broadcast(rtmp[:], rec_b[:])
                                nc.vector.tensor_copy(
                                    out=rec_all[D : 2 * D, :], in_=rtmp[:]
                                )
                        dst = ctxt_sb[:, fc, qsl]
                        nc.vector.tensor_mul(out=dst, in0=dst, in1=rec_all[:])

                        if fc == FC - 2 and qb == 1:
                            _xstack.close()
                            outpool_box[0] = _ostack.enter_context(
                                tc.tile_pool(name="outp", bufs=2)
                            )
                            partpool_box[0] = _ostack.enter_context(
                                tc.tile_pool(name="partp", bufs=8)
                            )

                # tail: just the fcc=7 finals for qb1's rows (sc 4..7)
                for sc in range(4, 8):
                    ot = outpool_box[0].tile([P, E], F32, tag="out", name="out")
                    for eb in range(2):
                        o_final(psum_pj, sc, eb, parts[sc], ot)
            _ostack.close()

    nc.compile()
    return nc


def _tile_wfc(wt):
    """Pre-tile W.T so chunk fc's 8 lhsT slices are one contiguous row-block:
    out[fc*128+p, ec*128+c] = wt[ec*128+p, fc*128+c]."""
    a = wt.reshape(EC, P, FC, P).transpose(2, 1, 0, 3)
    return np.ascontiguousarray(a.reshape(FC * P, E))


def _prep_inputs(X, Wq, Wk, Wv, Wo, bo):
    X = np.asarray(X, dtype=np.float32)
    wqt = np.ascontiguousarray(np.asarray(Wq, np.float32).T).astype(NPBF)
    wkt = np.ascontiguousarray(np.asarray(Wk, np.float32).T).astype(NPBF)
    wq2 = _tile_wfc(wqt)
    wk2 = _tile_wfc(wkt)
    wvt = np.ascontiguousarray(np.asarray(Wv, np.float32).T).astype(NPBF)
    wot = np.ascontiguousarray(np.asarray(Wo, np.float32).T).astype(NPBF)
    bo2 = np.ascontiguousarray(np.tile(np.asarray(bo, np.float32).reshape(1, E), (P, 1)))

    in_maps = []
    for c in range(NCORES):
        b, sh = c // 2, c % 2
        xt = np.ascontiguousarray(X[b].T)  # [E, S]
        if sh == 1:  # rotate so the local query half comes first
            xt = np.concatenate([xt[:, SL:], xt[:, :SL]], axis=1)
        in_maps.append(
            {
                "xt": np.ascontiguousarray(xt.astype(NPBF)),
                "wq2": wq2,
                "wk2": wk2,
                "wvt": wvt,
                "wot": wot,
                "bo": bo2,
            }
        )
    return in_maps


LAST_EXEC_NS = None
LAST_RESULTS = None


def _ensure_ntff_hook_importable():
    """bass_utils imports antenv.axon_hooks when tracing is requested (e.g.
    BASS_TRACE=1 in the environment).  The RL container's antenv stub lacks
    that module; register a no-op fallback so tracing degrades gracefully
    instead of crashing.  If a real antenv.axon_hooks exists, do nothing."""
    import sys
    import types

    try:
        import antenv.axon_hooks  # noqa: F401

        return
    except ImportError:
        pass
    try:
        import antenv

        mod = types.ModuleType("antenv.axon_hooks")
        _hook = [None]
        mod.set_axon_ntff_profile_hook = lambda h: _hook.__setitem__(0, h)
        mod.get_axon_ntff_profile_hook = lambda: _hook[0]
        sys.modules["antenv.axon_hooks"] = mod
        antenv.axon_hooks = mod
        try:
            from trn_agent_boot.trn_boot import _ntff_profile_via_ctypes

            mod.set_axon_ntff_profile_hook(
                _ntff_profile_via_ctypes("/opt/axon/libaxon_pjrt.so")
            )
        except Exception:
            pass
    except Exception:
        pass


def _run(in_maps, trace=False):
    global LAST_EXEC_NS, LAST_RESULTS
    _ensure_ntff_hook_importable()
    if "nc" not in _CACHE:
        _CACHE["nc"] = build()
    res = run_bass_kernel_spmd(
        _CACHE["nc"],
        in_maps,
        core_ids=list(range(NCORES)),
        trace=trace,
    )
    LAST_RESULTS = res
    LAST_EXEC_NS = res.exec_time_ns
    return res


def kernel(X, Wq, Wk, Wv, Wo, bo):
    in_maps = _prep_inputs(X, Wq, Wk, Wv, Wo, bo)
    res = _run(in_maps, trace=bool(int(os.environ.get("KERNEL_TRACE", "0"))))
    out = np.empty((B, S, E), np.float32)
    for c in range(NCORES):
        b, sh = c // 2, c % 2
        out[b, sh * SL : (sh + 1) * SL, :] = res.results[c]["out"]
    return out


# revision 31
# speedup vs baseline: 1.0311x; 1.0311x over previous
"""Multi-headed attention kernel for 8 Trainium2 NeuronCores.

Problem: B=4, S=2048, E=1024, H=16, D=64 (torch-convention Linears, no bias
on q/k/v, bias on output projection).

Sharding: core c handles (batch b = c//2, query half sh = c%2).  Each core
computes Q for its 1024 query rows, K/V for the full 2048 keys of its batch
(duplicated across the pair of cores sharing a batch -- cheaper than any
cross-core collective), all 16 heads of attention for its rows, and the
output projection + bias.  Zero collectives.

Layout (feature dim on partitions; scores computed transposed):
  qT[f, q]  = sum_e WqT[e, f] * XT[e, q]          kT[f, s] likewise
  V[s, f]   = sum_e XT[e, s-chunk] * WvT[e, f]    (natural layout)
  scoresT[k, q] = sum_d kT[h*64+d, kc] * qT[h*64+d, q]    (K=64 matmuls,
      head pair packed in complementary PE row groups)
  EX = exp(scoresT / 8)            (ACT engine, PSUM -> SBUF bf16)
  ctxT_aug[m, q] = sum_k Vaug[k, h*65+m] * EX[k, q]   m in 0..64; V carries
      a ones column per head, so row 64 of the accumulation = softmax
      denominators (ones-column trick, M=65 matmuls)
  ctxT_norm = ctxT * bcast(1/denom)   (recip on DVE over [1,512], then the
      idle GpSimd engine broadcasts it across partitions -- no PE involved)
  out[s, e] = sum_f ctxT_norm[f, s-chunk] * WoT[f, e] + bo   (bias is
      host-tiled to [128, E] and added by the DVE during the PSUM->SBUF
      copy -- no K=1 bias matmuls on the PE)

Scheduling notes (HW-trace driven):
  * LDWEIGHTS only ping-pongs into the background weight buffer when the
    tile config (size/position) matches the in-flight matmul; any config
    switch stalls ~100ns until the array drains.  The kc loop is therefore
    batched j=2: [pair kc, pair kc+1] then [ctx x4], so the tiled->full
    transitions amortize over two kc of work and same-config neighbors
    stream back-to-back at the N/2.4 rate.
  * The exp stream on the Scalar engine costs (1024+352)/1.2 = 1147ns per
    kc -- nearly co-critical with the PE.  The per-engine program order is
    fixed at compile time, so filler projections are explicitly interleaved
    between kc batches (one ~1.75us group slice per batch) instead of
    emitted in a blob at pair boundaries; the boundary blobs in the v1
    schedule produced 13us exp holes while ~50 queued fillers drained.
  * Softmax normalization runs immediately at unit end (den extraction and
    unnormalized-ctxt copy at high priority to release the ctx PSUM ring),
    with the 1/den broadcast on GpSimd instead of K=1 PE matmuls.
  PSUM: scores 2x[128,1024] (4) + ctx 2x[65,512] (2) + proj 2x[128,512]
  (2) = 8 banks.
"""

import os

import numpy as np
import ml_dtypes

import concourse.bass as bass
from concourse import bacc
import concourse.mybir as mybir
import concourse.tile as tile
from concourse.bass_utils import run_bass_kernel_spmd

B, S, E, H = 4, 2048, 1024, 16
D = E // H  # 64
P = 128
SL = S // 2     # local query rows per core (1024)
NCORES = 8
EC = E // P     # 8 e-chunks
FC = E // P     # 8 feature chunks
SC = S // P     # 16 s-chunks (V natural layout)
KC = S // P     # 16 key chunks (scores partition dim)
QB = SL // 512  # 2 query blocks of 512

F32 = mybir.dt.float32
BF16 = mybir.dt.bfloat16
EXPF = mybir.ActivationFunctionType.Exp
NPBF = ml_dtypes.bfloat16

_CACHE = {}


def build():
    nc = bacc.Bacc(
        "TRN2",
        target_bir_lowering=False,
        debug=False,
        num_devices=NCORES,
    )

    xt_d = nc.dram_tensor("xt", [E, S], BF16, kind="ExternalInput").ap()
    # wq2/wk2 are host-pretiled: row fc*128+p, col ec*128+c  =  W.T[ec*128+p,
    # fc*128+c], so one contiguous [128, E] DMA delivers all 8 lhsT slices
    # for feature chunk fc.
    wq2_d = nc.dram_tensor("wq2", [E, E], BF16, kind="ExternalInput").ap()
    wk2_d = nc.dram_tensor("wk2", [E, E], BF16, kind="ExternalInput").ap()
    wvt_d = nc.dram_tensor("wvt", [E, E], BF16, kind="ExternalInput").ap()
    wot_d = nc.dram_tensor("wot", [E, E], BF16, kind="ExternalInput").ap()
    bo_d = nc.dram_tensor("bo", [P, E], F32, kind="ExternalInput").ap()  # host-tiled bias
    out_d = nc.dram_tensor("out", [SL, E], F32, kind="ExternalOutput").ap()

    with tile.TileContext(nc) as tc:
     with tc.tile_pool(name="persist", bufs=1) as persist:
        qt_sb = persist.tile([P, FC, SL], BF16, tag="qt")
        kt_sb = persist.tile([P, FC, S], BF16, tag="kt")
        DA = D + 1  # head dim + ones column
        vaug_sb = persist.tile([P, SC, H * DA], BF16, tag="vaug")
        vview = vaug_sb.rearrange("p c (h d) -> p c h d", d=DA)
        nc.vector.memset(vview[:, :, :, D : D + 1], 1.0)
        ctxt_sb = persist.tile([P, FC, SL], BF16, tag="ctxt")

        ones_bf = persist.tile([1, P], BF16, tag="ones_bf")   # dummy-act input
        nc.vector.memset(ones_bf[:], 1.0)

        from contextlib import ExitStack

        with (
            tc.tile_pool(name="wvp", bufs=8) as wvpool,
            tc.tile_pool(name="wqkp", bufs=5) as wqkpool,
            tc.tile_pool(name="expp", bufs=10) as exppool,
            tc.tile_pool(name="smallp", bufs=4) as smallpool,
        ):
            _xstack = ExitStack()
            _ostack = ExitStack()
            xpool = _xstack.enter_context(tc.tile_pool(name="xp", bufs=1))
            x_sb = xpool.tile([P, EC, S], BF16, tag="x")
            wv = []
            wot_t = []
            bo_sb = persist.tile([P, E], F32, tag="bo")
            outpool_box = [None]

            def load_wfc(w_dram, fc):
                """One [128, E] tile holding all 8 lhsT slices for chunk fc."""
                t = wqkpool.tile([P, E], BF16, tag="wqk", name="wqk")
                nc.sync.dma_start(out=t[:], in_=w_dram[fc * P : (fc + 1) * P, :])
                return t

            # ---- projection group emitters (8 accumulating MMs + 1 cast) ----
            def q_group(pool, wq_t, fc, qb):
                ps = pool.tile([P, 512], F32, tag="pj", name="pj")
                for ec in range(EC):
                    nc.tensor.matmul(
                        ps[:],
                        wq_t[:, ec * P : (ec + 1) * P],
                        x_sb[:, ec, qb * 512 : qb * 512 + 512],
                        start=(ec == 0),
                        stop=(ec == EC - 1),
                    )
                nc.vector.tensor_copy(
                    out=qt_sb[:, fc, qb * 512 : qb * 512 + 512], in_=ps[:]
                )

            def k_group(pool, wk_t, fc, kb):
                ps = pool.tile([P, 512], F32, tag="pj", name="pj")
                for ec in range(EC):
                    nc.tensor.matmul(
                        ps[:],
                        wk_t[:, ec * P : (ec + 1) * P],
                        x_sb[:, ec, kb * 512 : kb * 512 + 512],
                        start=(ec == 0),
                        stop=(ec == EC - 1),
                    )
                nc.vector.tensor_copy(
                    out=kt_sb[:, fc, kb * 512 : kb * 512 + 512], in_=ps[:]
                )

            def v_group(pool, sc, fb):
                ps = pool.tile([P, 512], F32, tag="pj", name="pj")
                for ec in range(EC):
                    nc.tensor.matmul(
                        ps[:],
                        x_sb[:, ec, sc * P : (sc + 1) * P],
                        wv[ec][:, fb * 512 : fb * 512 + 512],
                        start=(ec == 0),
                        stop=(ec == EC - 1),
                    )
                vv = vaug_sb[:, sc, :].rearrange("p (h d) -> p h d", d=DA)
                nc.vector.tensor_copy(
                    out=vv[:, fb * 8 : (fb + 1) * 8, 0:D],
                    in_=ps.rearrange("p (h d) -> p h d", d=D),
                )

            def o_group(pool, sc, eb, ot):
                ps = pool.tile([P, 512], F32, tag="pj", name="pj")
                for fcc in range(FC):
                    nc.tensor.matmul(
                        ps[:],
                        ctxt_sb[:, fcc, sc * P : (sc + 1) * P],
                        wot_t[fcc][:, eb * 512 : eb * 512 + 512],
                        start=(fcc == 0),
                        stop=(fcc == FC - 1),
                    )
                # bias folded into the PSUM->SBUF copy on the DVE (the bias
                # row is host-tiled across partitions), dropping the 16 K=1
                # bias matmuls and their tile-config switch stalls
                nc.vector.scalar_tensor_tensor(
                    out=ot[:, eb * 512 : eb * 512 + 512],
                    in0=ps[:],
                    scalar=1.0,
                    in1=bo_sb[:, eb * 512 : eb * 512 + 512],
                    op0=mybir.AluOpType.mult,
                    op1=mybir.AluOpType.add,
                )
                if eb == 1:
                    nc.sync.dma_start(
                        out=out_d[sc * P : (sc + 1) * P, :], in_=ot[:]
                    )

            # ---------------- upfront: just enough for pair 0 ----------------
            # W chunk-0 tiles go on the Sync DMA queue; X streams across the
            # GpSimd/Scalar/Vector DMA queues, first-half (hx=0) chunks
            # first so the qb0/kb0-1 projection groups complete after 8
            # transfers instead of 15; Wv after X on Sync.
            wq_sl = load_wfc(wq2_d, 0)
            wk_sl = load_wfc(wk2_d, 0)
            for hx in range(2):
                for ec in range(EC):
                    eng = nc.gpsimd if ec % 2 == 0 else nc.scalar
                    eng.dma_start(
                        out=x_sb[:, ec, hx * 1024 : (hx + 1) * 1024],
                        in_=xt_d[ec * P : (ec + 1) * P, hx * 1024 : (hx + 1) * 1024],
                    )
            # Dummy exp preloads the ACT table (~2.7us) during the DMA phase;
            # emitted AFTER the X dma_starts so it doesn't delay the Scalar
            # queue's descriptor generation.
            dummy_act = smallpool.tile([1, 16], BF16, tag="recb", name="recb")
            nc.scalar.activation(dummy_act[:], ones_bf[0:1, 0:16], EXPF)
            for ec in range(EC):
                t = wvpool.tile([P, E], BF16, tag="wv", name="wv")
                nc.sync.dma_start(out=t[:], in_=wvt_d[ec * P : (ec + 1) * P, :])
                wv.append(t)
            with tc.tile_pool(name="psum_u", bufs=6, space="PSUM") as psum_u:
                # advance all 6 Q/K accumulation groups together per arriving
                # X chunk: each 1.6us chunk DMA feeds ~1.6us of matmuls, so
                # the PE ramps with the DMA stream instead of stalling on the
                # last chunk of each group.
                psq = [
                    psum_u.tile([P, 512], F32, tag="pj", name="pj")
                    for _ in range(QB)
                ]
                psk = [
                    psum_u.tile([P, 512], F32, tag="pj", name="pj")
                    for _ in range(4)
                ]
                # wave 1: the four groups needing only first-half X chunks
                # (qb0/qb1/kb0/kb1 all live in columns 0:1024), interleaved
                # per arriving hx=0 chunk; kb2/kb3 form wave 2 behind the
                # hx=1 transfers so the compile-time PE stream never stalls
                # on a second-half chunk mid-wave.
                for ec in range(EC):
                    for qb in range(QB):
                        nc.tensor.matmul(
                            psq[qb][:],
                            wq_sl[:, ec * P : (ec + 1) * P],
                            x_sb[:, ec, qb * 512 : qb * 512 + 512],
                            start=(ec == 0),
                            stop=(ec == EC - 1),
                        )
                    for kb in range(2):
                        nc.tensor.matmul(
                            psk[kb][:],
                            wk_sl[:, ec * P : (ec + 1) * P],
                            x_sb[:, ec, kb * 512 : kb * 512 + 512],
                            start=(ec == 0),
                            stop=(ec == EC - 1),
                        )
                for qb in range(QB):
                    nc.vector.tensor_copy(
                        out=qt_sb[:, 0, qb * 512 : qb * 512 + 512], in_=psq[qb][:]
                    )
                for kb in range(2):
                    nc.vector.tensor_copy(
                        out=kt_sb[:, 0, kb * 512 : kb * 512 + 512], in_=psk[kb][:]
                    )
                # V first (hx=0-only deps) so wave 2 overlaps the hx=1 DMAs
                for sc in range(4):
                    v_group(psum_u, sc, 0)
                for sc in range(4, 6):
                    v_group(psum_u, sc, 0)
                # wave 2: kb2/kb3 (second-half X chunks)
                for ec in range(EC):
                    for kb in range(2, 4):
                        nc.tensor.matmul(
                            psk[kb][:],
                            wk_sl[:, ec * P : (ec + 1) * P],
                            x_sb[:, ec, kb * 512 : kb * 512 + 512],
                            start=(ec == 0),
                            stop=(ec == EC - 1),
                        )
                for kb in range(2, 4):
                    nc.vector.tensor_copy(
                        out=kt_sb[:, 0, kb * 512 : kb * 512 + 512], in_=psk[kb][:]
                    )

            # ---------------- main loop: (fc, qb) units, kc batches of 2 ----
            with (
                tc.tile_pool(name="psum_sc", bufs=2, space="PSUM") as psum_sc,
                tc.tile_pool(name="psum_cx", bufs=2, space="PSUM") as psum_cx,
                tc.tile_pool(name="psum_pj", bufs=2, space="PSUM") as psum_pj,
            ):
                # ---- static filler schedule: unit (fc,qb) -> list of
                # closures, one emitted after each kc batch (8 slots/unit).
                # K(f)/Q(f) prepped during fc=f-1; V fb0 upfront+unit(0,0);
                # V fb1 during fc in 1..3; out-proj sc0-3 during (7,1).
                wnames = {}

                def _load_k(f):
                    def go():
                        wnames[("k", f)] = load_wfc(wk2_d, f)
                    return go

                def _load_q(f):
                    def go():
                        wnames[("q", f)] = load_wfc(wq2_d, f)
                    return go

                def _k(f, kb):
                    def go():
                        k_group(psum_pj, wnames[("k", f)], f, kb)
                    return go

                def _q(f, qb):
                    def go():
                        q_group(psum_pj, wnames[("q", f)], f, qb)
                    return go

                def _v(sc, fb):
                    def go():
                        v_group(psum_pj, sc, fb)
                    return go

                def _wo_prefetch():
                    def go():
                        for fcc in range(FC):
                            t = wvpool.tile([P, E], BF16, tag="wv", name="wv")
                            nc.sync.dma_start(
                                out=t[:], in_=wot_d[fcc * P : (fcc + 1) * P, :]
                            )
                            wot_t.append(t)
                        nc.sync.dma_start(out=bo_sb[:], in_=bo_d[:])
                    return go

                def _o(sc, eb, ot_box):
                    def go():
                        if ot_box[0] is None or eb == 0:
                            ot_box[0] = outpool_box[0].tile(
                                [P, E], F32, tag="out", name="out"
                            )
                        o_group(psum_pj, sc, eb, ot_box[0])
                    return go

                sched = {}
                for fc in range(FC):
                    for qb in range(QB):
                        sched[(fc, qb)] = []
                u = sched
                # unit (0,0): V sc6..15 inline (2 per batch, 4-chunk lead),
                # then K(1) kb0/kb1 + loads
                for b in range(5):
                    u[(0, 0)] += [_v(2 * b + 6, 0), _v(2 * b + 7, 0)]
                u[(0, 0)] += [_load_k(1), _k(1, 0), _load_q(1), _k(1, 1)]
                # unit (0,1): Q(1,*) + K(1) kb2/3 + start V fb1
                u[(0, 1)] += [
                    _q(1, 0), _k(1, 2), _k(1, 3), _q(1, 1),
                    _v(0, 1), _v(1, 1), _v(2, 1), _v(3, 1),
                ]
                vn = 4  # next fb1 V chunk
                for fc in range(1, FC - 1):
                    nf = fc + 1
                    a = [_load_k(nf), _k(nf, 0), _load_q(nf), _k(nf, 1)]
                    bl = [_q(nf, 0), _k(nf, 2), _k(nf, 3), _q(nf, 1)]
                    if fc < 4:
                        a += [_v(vn, 1), _v(vn + 1, 1)]
                        bl += [_v(vn + 2, 1), _v(vn + 3, 1)]
                        vn += 4
                    if fc == 4:
                        a = [_wo_prefetch()] + a
                    u[(fc, 0)] += a
                    u[(fc, 1)] += bl
                # unit (7,1): out-proj for qb0's rows (sc 0..3)
                ot_boxes = [[None] for _ in range(8)]
                for sc in range(4):
                    u[(7, 1)] += [_o(sc, 0, ot_boxes[sc]), _o(sc, 1, ot_boxes[sc])]

                for fc in range(FC):
                    hA, hB = 2 * fc, 2 * fc + 1
                    for qb in range(QB):
                        fill = list(sched[(fc, qb)])
                        ctx_ps = {
                            hh: psum_cx.tile([DA, 512], F32, tag="ctx", name="ctx")
                            for hh in (0, 1)
                        }
                        for kcb in range(KC // 2):
                            with tc.high_priority(offset=1 << 20):
                                exs = []
                                for j2 in (0, 1):
                                    kc = 2 * kcb + j2
                                    sc_ps = psum_sc.tile(
                                        [P, 1024], F32, tag="sc", name="sc"
                                    )
                                    for hh in (0, 1):
                                        po = hh * D
                                        nc.tensor.matmul(
                                            sc_ps[:, hh * 512 : hh * 512 + 512],
                                            kt_sb[
                                                po : po + D,
                                                fc,
                                                kc * P : (kc + 1) * P,
                                            ],
                                            qt_sb[
                                                po : po + D,
                                                fc,
                                                qb * 512 : qb * 512 + 512,
                                            ],
                                            start=True,
                                            stop=True,
                                        )
                                    ex = exppool.tile(
                                        [P, 1024], BF16, tag="exp", name="exp"
                                    )
                                    nc.scalar.activation(
                                        ex[:], sc_ps[:], EXPF, scale=0.125
                                    )
                                    exs.append(ex)
                                for j2 in (0, 1):
                                    kc = 2 * kcb + j2
                                    for hh, h in ((0, hA), (1, hB)):
                                        nc.tensor.matmul(
                                            ctx_ps[hh][0:DA, :],
                                            vaug_sb[:, kc, h * DA : (h + 1) * DA],
                                            exs[j2][:, hh * 512 : hh * 512 + 512],
                                            start=(kc == 0),
                                            stop=(kc == KC - 1),
                                        )
                            # filler slices (normal priority): drain the
                            # unit's queue evenly across its 8 batches
                            nb = KC // 2 - kcb
                            npop = (len(fill) + nb - 1) // nb if fill else 0
                            for _ in range(npop):
                                fill.pop(0)()

                        # ---- normalize immediately: den + unnorm copy at
                        # high priority (releases the cx ring), recip/bcast/
                        # mul off the PE.
                        qsl = slice(qb * 512, qb * 512 + 512)
                        dens = []
                        with tc.high_priority(offset=1 << 20):
                            for hh in (0, 1):
                                den = smallpool.tile(
                                    [1, 512], F32, tag="den", name="den"
                                )
                                nc.vector.tensor_copy(
                                    out=den[:], in_=ctx_ps[hh][D : D + 1, :]
                                )
                                dens.append(den)
                            for hh in (0, 1):
                                nc.vector.tensor_copy(
                                    out=ctxt_sb[hh * D : (hh + 1) * D, fc, qsl],
                                    in_=ctx_ps[hh][0:D, :],
                                )
                        rec_all = smallpool.tile(
                            [P, 512], BF16, tag="recall", name="recall"
                        )
                        for hh in (0, 1):
                            rec_f = smallpool.tile(
                                [1, 512], F32, tag="recf", name="recf"
                            )
                            nc.vector.reciprocal_approx_fast(
                                out=rec_f[:], in_=dens[hh][:]
                            )
                            rec_b = smallpool.tile(
                                [1, 512], BF16, tag="recb", name="recb"
                            )
                            nc.vector.tensor_copy(out=rec_b[:], in_=rec_f[:])
                            # partition_broadcast's write mask uses absolute
                            # partition indices (cpu_id*16+lane < channels),
                            # so it can only target base-partition-0 slices;
                            # head B goes through a base-0 tile + DVE copy.
                            if hh == 0:
                                nc.gpsimd.partition_broadcast(
                                    rec_all[0:D, :], rec_b[:]
                                )
                            else:
                                rtmp = smallpool.tile(
                                    [D, 512], BF16, tag="rtmp", name="rtmp"
                                )
                                nc.gpsimd.partition_broadcast(rtmp[:], rec_b[:])
                                nc.vector.tensor_copy(
                                    out=rec_all[D : 2 * D, :], in_=rtmp[:]
                                )
                        dst = ctxt_sb[:, fc, qsl]
                        nc.vector.tensor_mul(out=dst, in0=dst, in1=rec_all[:])

                        if fc == FC - 2 and qb == 1:
                            _xstack.close()
                            outpool_box[0] = _ostack.enter_context(
                                tc.tile_pool(name="outp", bufs=2)
                            )

                # tail: out-proj for qb1's rows (sc 4..7)
                for sc in range(4, 8):
                    ot = outpool_box[0].tile([P, E], F32, tag="out", name="out")
                    for eb in range(2):
                        o_group(psum_pj, sc, eb, ot)
            _ostack.close()

    nc.compile()
    return nc


def _tile_wfc(wt):
    """Pre-tile W.T so chunk fc's 8 lhsT slices are one contiguous row-block:
    out[fc*128+p, ec*128+c] = wt[ec*128+p, fc*128+c]."""
    a = wt.reshape(EC, P, FC, P).transpose(2, 1, 0, 3)
    return np.ascontiguousarray(a.reshape(FC * P, E))


def _prep_inputs(X, Wq, Wk, Wv, Wo, bo):
    X = np.asarray(X, dtype=np.float32)
    wqt = np.ascontiguousarray(np.asarray(Wq, np.float32).T).astype(NPBF)
    wkt = np.ascontiguousarray(np.asarray(Wk, np.float32).T).astype(NPBF)
    wq2 = _tile_wfc(wqt)
    wk2 = _tile_wfc(wkt)
    wvt = np.ascontiguousarray(np.asarray(Wv, np.float32).T).astype(NPBF)
    wot = np.ascontiguousarray(np.asarray(Wo, np.float32).T).astype(NPBF)
    bo2 = np.ascontiguousarray(np.tile(np.asarray(bo, np.float32).reshape(1, E), (P, 1)))

    in_maps = []
    for c in range(NCORES):
        b, sh = c // 2, c % 2
        xt = np.ascontiguousarray(X[b].T)  # [E, S]
        if sh == 1:  # rotate so the local query half comes first
            xt = np.concatenate([xt[:, SL:], xt[:, :SL]], axis=1)
        in_maps.append(
            {
                "xt": np.ascontiguousarray(xt.astype(NPBF)),
                "wq2": wq2,
                "wk2": wk2,
                "wvt": wvt,
                "wot": wot,
                "bo": bo2,
            }
        )
    return in_maps


LAST_EXEC_NS = None
LAST_RESULTS = None


def _ensure_ntff_hook_importable():
    """bass_utils imports antenv.axon_hooks when tracing is requested (e.g.
    BASS_TRACE=1 in the environment).  The RL container's antenv stub lacks
    that module; register a no-op fallback so tracing degrades gracefully
    instead of crashing.  If a real antenv.axon_hooks exists, do nothing."""
    import sys
    import types

    try:
        import antenv.axon_hooks  # noqa: F401

        return
    except ImportError:
        pass
    try:
        import antenv

        mod = types.ModuleType("antenv.axon_hooks")
        _hook = [None]
        mod.set_axon_ntff_profile_hook = lambda h: _hook.__setitem__(0, h)
        mod.get_axon_ntff_profile_hook = lambda: _hook[0]
        sys.modules["antenv.axon_hooks"] = mod
        antenv.axon_hooks = mod
        try:
            from trn_agent_boot.trn_boot import _ntff_profile_via_ctypes

            mod.set_axon_ntff_profile_hook(
                _ntff_profile_via_ctypes("/opt/axon/libaxon_pjrt.so")
            )
        except Exception:
            pass
    except Exception:
        pass


def _run(in_maps, trace=False):
    global LAST_EXEC_NS, LAST_RESULTS
    _ensure_ntff_hook_importable()
    if "nc" not in _CACHE:
        _CACHE["nc"] = build()
    res = run_bass_kernel_spmd(
        _CACHE["nc"],
        in_maps,
        core_ids=list(range(NCORES)),
        trace=trace,
    )
    LAST_RESULTS = res
    LAST_EXEC_NS = res.exec_time_ns
    return res


def kernel(X, Wq, Wk, Wv, Wo, bo):
    in_maps = _prep_inputs(X, Wq, Wk, Wv, Wo, bo)
    res = _run(in_maps, trace=bool(int(os.environ.get("KERNEL_TRACE", "0"))))
    out = np.empty((B, S, E), np.float32)
    for c in range(NCORES):
        b, sh = c // 2, c % 2
        out[b, sh * SL : (sh + 1) * SL, :] = res.results[c]["out"]
    return out
